# revision 27
# baseline (speedup 1.0000x reference)
"""Trainium2 Bass kernel for nn_CGNN_83605833384509.

Banded-DAG CGNN: gen[:, n] = MLP_n(gen[:, n-4:n] masked, noise[:, n]),
n = 0..63 sequential, B = 262144 batch.

Device strategy (unchanged from the working pipeline): data-parallel
over 8 cores (B/8 = 32768 each). Per core, a node-staggered software
pipeline ("superwaves"): at superwave s node n processes chunk c = s - n
(chunks of W=512 columns). Generated values live in a windowed,
partition-replicated SBUF ring tensor X so every matmul reads/writes
32-aligned partition windows. Per node: z = W1g.gen_parents + W1n.noise
+ b1 via accumulating 32x32-tile matmuls (3 nodes packed per matmul),
relu via ACT/DVE psum->SBUF evacuation, y = W2.h + b2 via embedded-
column matmuls, all active nodes' y written back to X in one 128-lane
op. Noise streams in / gen streams out via diagonal-in-DRAM DMA.

Host strategy (the part that dominates wall time — the axon tunnel runs
at ~60 MB/s shared both ways): ONE per-core DRAM input `nin`[80,32768]
bf16 packing the 64 noise rows plus the packed weights (16 rows), so
the whole upload is a single 40MB put to device 0 followed by a
terminal-side scatter (device-to-device, ~0.1s). SBUF state (X ring,
noise ring, bias-ones rows) is initialized with on-device memsets
instead of DMAing 32MB of literal zeros from the host. The donated
output buffers are recycled device arrays from the previous call (no
wire traffic). The built+compiled Bass module and the jitted sharded
executable are cached at module level, so repeat calls skip tracing and
compilation entirely. Output is fetched with 8 concurrent per-shard
reads (the tunnel serves parallel gets ~3x faster than one stream).
"""

import os
import time
import threading
import numpy as np

# ---------------------------------------------------------------- constants
NN = 64          # nodes
KP = 4           # max parents
NH = 10          # hidden width
W = 512          # chunk width (psum bank = 512 fp32)
C = 64           # chunks per core: B_shard = C*W = 32768
B_SHARD = C * W
N_CORES = 8
B_FULL = B_SHARD * N_CORES
NSTREAM = 4               # independent chunk-range streams (pipeline overlap)
CS = C // NSTREAM         # chunks per stream
NSW = CS + NN - 1         # superwaves per stream
XRING = 32                # gen ring slots total (16 per stream)
XR_S = XRING // NSTREAM
NRING = 16                # noise ring slots total (8 per stream)
NR_S = NRING // NSTREAM
NLAG = 2                  # noise refresh lead (superwaves), < NR_S
HQ = 2                    # Hbuf ring depth per stream
NZB = 6                   # z psum banks

# Packed single-input layout: rows 0..63 = noiseT, rows 64.. = a flat
# weight region: wph compact [32, 3300] (phase blocks only ever occupy
# one 32-row quadrant), wl2 [128, 768], b2c [128, 1], then an 8192-long
# ones vector for the bias rows of XN (DMA can write single partitions;
# DVE memset cannot).
WTS_COLS = 0              # filled below once NTRIO known
NIN_WROWS = 7             # weight+ones rows appended after the noise rows
NIN_ROWS = NN + NIN_WROWS

# Windows: quadrant q holds gen rows for nodes [wlo, whi] at partition
# 32*q + (m - wlo).  Every trio's parents+self fit in its own window.
WIN = [(0, 14), (8, 29), (24, 45), (40, 63)]
NTRIO = 22
PH_COLS = NTRIO * 5 * 30                    # 3300
L2_COLS = NZB * 128                         # 768
WTS_COLS = PH_COLS + L2_COLS + 1            # SBUF tile: wph | wl2 | b2c
# flat element offsets inside the weight region (after the noise rows)
WOFF_PH = 0                                 # [32, 3300] quadrant-compact
WOFF_L2 = WOFF_PH + 32 * PH_COLS            # [128, 768]
WOFF_B2 = WOFF_L2 + 128 * L2_COLS           # [128, 1]
ONES_LEN = NRING * W
WOFF_ONES = WOFF_B2 + 128                   # [ONES_LEN]
WREG_LEN = WOFF_ONES + ONES_LEN
assert WREG_LEN <= NIN_WROWS * B_SHARD
# wph column spans per quadrant (trio_win is monotone in tau)
PH_SPANS = [(0, 0, 750), (1, 750, 1500), (2, 1500, 2250), (3, 2250, 3300)]


def trio_nodes(tau):
    return [n for n in range(3 * tau, min(3 * tau + 3, NN))]


def trio_win(tau):
    n0 = 3 * tau
    if n0 <= 12:
        return 0
    if n0 <= 27:
        return 1
    if n0 <= 42:
        return 2
    return 3


def win_rows(q):
    lo, hi = WIN[q]
    return hi - lo + 1


def pos_in_win(m, q):
    """partition row of gen-node m inside window q (must be present)."""
    lo, hi = WIN[q]
    assert lo <= m <= hi, (m, q)
    return 32 * q + (m - lo)


def windows_of(m):
    return [q for q in range(4) if WIN[q][0] <= m <= WIN[q][1]]


# z-psum placement: trio tau -> (zq, zb): quadrant zq = tau % 4, bank
# zb = tau // 4 (6 banks).  z rows = 32*zq .. 32*zq+29 (3 nodes x 10).
def trio_zq(tau):
    return tau % 4


def trio_zb(tau):
    return tau // 4


def active_range(s):
    return max(0, s - CS + 1), min(NN - 1, s)


def trio_active(tau, s):
    lo, hi = active_range(s)
    ns = trio_nodes(tau)
    return ns[0] <= hi and ns[-1] >= lo


# ------------------------------------------------------------- weight packing
def w1_row_for_parent(n, j):
    """W1 slot row holding the weight of parent m = n - j for node n."""
    if n >= KP:
        return KP - j
    return n - j  # left-aligned parents for n < 4


def pack_weights(W1, b1, W2, b2):
    """Build the packed flat weight region [WREG_LEN] (f32) laid out as
    wph-compact [32, 3300] | wl2 [128, 768] | b2c [128, 1] | ones.
    Also returns phase_nz [NTRIO][5] (static given the banded DAG).
    """
    W1 = np.asarray(W1, np.float32)
    b1 = np.asarray(b1, np.float32)
    W2 = np.asarray(W2, np.float32)
    b2 = np.asarray(b2, np.float32)

    wts = np.zeros((128, WTS_COLS), np.float32)
    wph = wts[:, :PH_COLS]
    wl2 = wts[:, PH_COLS:PH_COLS + L2_COLS]
    b2c = wts[:, WTS_COLS - 1:WTS_COLS]

    phase_nz = np.zeros((NTRIO, 5), bool)
    for tau in range(NTRIO):
        q = trio_win(tau)
        for j in range(5):
            off = (tau * 5 + j) * 30
            blk = wph[:, off:off + 30]
            for i, n in enumerate(trio_nodes(tau)):
                if j == 0:
                    # noise weights at node's own row + bias on ones-row 31
                    blk[pos_in_win(n, q), 10 * i:10 * i + 10] = W1[n, KP]
                    blk[32 * q + 31, 10 * i:10 * i + 10] = b1[n]
                    phase_nz[tau, j] = True
                else:
                    m = n - j
                    if m < 0:
                        continue
                    blk[pos_in_win(m, q), 10 * i:10 * i + 10] = \
                        W1[n, w1_row_for_parent(n, j)]
                    phase_nz[tau, j] = True

    # L2: one full-array (128 x 128) lhsT per z-bank: contracts the bank's
    # whole Hbuf column (its 4 trios), writes y at every window position of
    # its nodes (zero columns elsewhere); banks accumulate into y psum.
    for zb in range(NZB):
        blk = wl2[:, zb * 128:(zb + 1) * 128]
        for t in range(zb * 4, min(zb * 4 + 4, NTRIO)):
            zq = trio_zq(t)
            for i, n in enumerate(trio_nodes(t)):
                for oq in windows_of(n):
                    blk[32 * zq + 10 * i:32 * zq + 10 * i + 10,
                        pos_in_win(n, oq)] = W2[n]

    for m in range(NN):
        for q in windows_of(m):
            b2c[pos_in_win(m, q), 0] = b2[m]

    wreg = np.empty((WREG_LEN,), np.float32)
    phc = wreg[WOFF_PH:WOFF_L2].reshape(32, PH_COLS)
    for q, c0, c1 in PH_SPANS:
        phc[:, c0:c1] = wph[32 * q:32 * q + 32, c0:c1]
    wreg[WOFF_L2:WOFF_B2] = wl2.ravel()
    wreg[WOFF_B2:WOFF_ONES] = b2c.ravel()
    wreg[WOFF_ONES:] = 1.0
    return wreg, phase_nz


_PHASE_NZ = None  # filled by pack_weights caller before build_bass


# ------------------------------------------------------------- schedule
def xn_dma_jobs(sp):
    """Noise-refresh DMA jobs for superwave sp: list of
    (quad, row_a, nrows, n_lo, ring_slot, c_lo).  SBUF rows row_a.. get
    noise rows n_lo.. at chunk offsets c = sp - n (linear in n)."""
    lo, hi = active_range(sp)
    jobs = []
    by_q = {}
    for n in range(lo, hi + 1):
        q = trio_win(n // 3)
        by_q.setdefault(q, []).append(n)
    for q, ns in sorted(by_q.items()):
        n_lo, n_hi = ns[0], ns[-1]
        assert ns == list(range(n_lo, n_hi + 1))
        row_a = pos_in_win(n_lo, q)
        jobs.append((q, row_a, n_hi - n_lo + 1, n_lo, sp % NR_S, sp - n_lo))
    return jobs


def out_dma_jobs(sg):
    """Gen DMA-out jobs for slot written at superwave sg: list of
    (quad, row_a, nrows, m_lo, ring_slot, c_lo)."""
    lo, hi = active_range(sg)
    jobs = []
    bounds = [(0, 14), (15, 29), (30, 45), (46, 63)]
    for q, (plo, phi) in enumerate(bounds):
        m_lo, m_hi = max(lo, plo), min(hi, phi)
        if m_lo > m_hi:
            continue
        row_a = pos_in_win(m_lo, q)
        jobs.append((q, row_a, m_hi - m_lo + 1, m_lo, sg % XR_S, sg - m_lo))
    return jobs


# ------------------------------------------------------------- bass kernel
def build_bass(phase_nz, w=W, c=C):
    import concourse.bass as bass
    import concourse.bacc as bacc
    import concourse.mybir as mybir
    import concourse.tile as tile

    f32 = mybir.dt.float32
    bf16 = mybir.dt.bfloat16
    RELU = mybir.ActivationFunctionType.Relu

    nc = bacc.Bacc("TRN2", target_bir_lowering=False, debug=False,
                   enable_asserts=False, num_devices=N_CORES)

    # Single packed input: rows 0..63 noiseT, then the flat weight
    # region.  Output split in two tensors (nodes 0-31 / 32-63) so the
    # host can fetch over 16 concurrent streams.
    d_nin = nc.dram_tensor("nin", [NIN_ROWS, c * w], bf16,
                           kind="ExternalInput").ap()
    d_gen = [nc.dram_tensor("gen0", [NN // 2, c * w], bf16,
                            kind="ExternalOutput").ap(),
             nc.dram_tensor("gen1", [NN // 2, c * w], bf16,
                            kind="ExternalOutput").ap()]

    with tile.TileContext(nc) as tc:
        with tc.tile_pool(name="sb", bufs=1) as sb, \
             tc.tile_pool(name="ps", bufs=1, space="PSUM") as pp:
            cs = c // NSTREAM
            nsw = cs + NN - 1
            X = sb.tile([128, XRING * w], bf16)
            XN = sb.tile([128, NRING * w], bf16)
            Hbuf = sb.tile([128, NSTREAM * HQ * NZB * w], bf16)
            WTS = sb.tile([128, WTS_COLS], bf16)
            B2C = sb.tile([128, 1], f32)
            zpt = [pp.tile([128, 2 * w], f32, name=f"zpt{i}")
                   for i in range(NZB // 2)]
            yps = [pp.tile([128, w], f32, name=f"yps{i}") for i in range(2)]

            WPH_OFF = 0
            WL2_OFF = PH_COLS
            base = NN * c * w

            # weights: quadrant-compact wph + wl2 + b2c from the packed
            # input rows
            for q, c0, c1 in PH_SPANS:
                src = bass.AP(d_nin.tensor, base + WOFF_PH + c0,
                              [[PH_COLS, 32], [1, c1 - c0]])
                nc.sync.dma_start(WTS[32 * q:32 * q + 32, c0:c1], src)
            src = bass.AP(d_nin.tensor, base + WOFF_L2,
                          [[L2_COLS, 128], [1, L2_COLS]])
            nc.sync.dma_start(WTS[:, PH_COLS:PH_COLS + L2_COLS], src)
            src = bass.AP(d_nin.tensor, base + WOFF_B2, [[1, 128], [1, 1]])
            nc.sync.dma_start(WTS[:, WTS_COLS - 1:WTS_COLS], src)
            # b2c column as f32 for the y-evacuation add
            nc.scalar.copy(B2C[:], WTS[:, WTS_COLS - 1:WTS_COLS])

            # SBUF state init on device (replaces 32MB of zeros DMA)
            nc.vector.memset(X[:], 0.0)
            nc.vector.memset(XN[:], 0.0)
            for qi in range(4):
                osrc = bass.AP(d_nin.tensor, base + WOFF_ONES,
                               [[ONES_LEN, 1], [1, ONES_LEN]])
                nc.sync.dma_start(XN[32 * qi + 31:32 * qi + 32, :], osrc)
            for t in zpt:
                nc.vector.memset(t[:], 0.0)
            for t in yps:
                nc.vector.memset(t[:], 0.0)

            def xn_refresh(sg, sp):
                if sp >= nsw:
                    return
                cb = sg * cs
                for (q, row_a, nrows, n_lo, rs, c_lo) in xn_dma_jobs(sp):
                    k_ok = [k for k in range(nrows) if 0 <= c_lo - k < cs]
                    if not k_ok:
                        continue
                    k0, k1 = min(k_ok), max(k_ok)
                    off = (n_lo + k0) * c * w + (cb + c_lo - k0) * w
                    src_ap = bass.AP(d_nin.tensor, off,
                                     [[c * w - w, k1 - k0 + 1], [1, w]])
                    sl = sg * NR_S + rs
                    nc.sync.dma_start(
                        XN[row_a + k0:row_a + k1 + 1, sl * w:(sl + 1) * w],
                        src_ap)

            def dma_out(sg, so):
                cb = sg * cs
                for (q, row_a, nrows, m_lo, rs, c_lo) in out_dma_jobs(so):
                    sl = sg * XR_S + rs
                    # split at node 32 (output tensor boundary)
                    for (t_id, p_lo, p_hi) in ((0, m_lo,
                                                min(m_lo + nrows - 1, 31)),
                                               (1, max(m_lo, 32),
                                                m_lo + nrows - 1)):
                        if p_lo > p_hi:
                            continue
                        d = p_lo - m_lo
                        off = ((p_lo - 32 * t_id) * c * w
                               + (cb + c_lo - d) * w)
                        dst = bass.AP(d_gen[t_id].tensor, off,
                                      [[c * w - w, p_hi - p_lo + 1], [1, w]])
                        nc.sync.dma_start(
                            dst, X[row_a + d:row_a + d + p_hi - p_lo + 1,
                                   sl * w:(sl + 1) * w])

            for sg in range(NSTREAM):
                for sp in range(min(NLAG, nsw)):
                    xn_refresh(sg, sp)

            for t in range(nsw):
                for sg in range(NSTREAM):
                    s = t
                    xn_refresh(sg, s + NLAG)
                    act_trios = [tt for tt in range(NTRIO)
                                 if trio_active(tt, s)]
                    for tau in act_trios:
                        q, zq, zb = trio_win(tau), trio_zq(tau), trio_zb(tau)
                        js = [j for j in (0, 4, 3, 2, 1) if phase_nz[tau, j]]
                        for ji, j in enumerate(js):
                            off = WPH_OFF + (tau * 5 + j) * 30
                            if j == 0:
                                kw = 32
                                sl = sg * NR_S + (s % NR_S)
                                rhs = XN[32 * q:32 * q + 32,
                                         sl * w:(sl + 1) * w]
                            else:
                                kw = win_rows(q)
                                sl = sg * XR_S + ((s - j) % XR_S)
                                rhs = X[32 * q:32 * q + kw,
                                        sl * w:(sl + 1) * w]
                            lhsT = WTS[32 * q:32 * q + kw, off:off + 30]
                            nc.tensor.matmul(
                                zpt[zb // 2][32 * zq:32 * zq + 30,
                                             (zb % 2) * w:(zb % 2) * w + w],
                                lhsT, rhs,
                                start=(ji == 0), stop=(ji == len(js) - 1),
                                skip_group_check=True,
                                tile_position=(32 * q, 32 * zq))
                    act_banks0 = sorted({trio_zb(tt) for tt in act_trios})
                    act_pairs = sorted({zb // 2 for zb in act_banks0})
                    for bi, pb in enumerate(act_pairs):
                        hcol = (((sg * HQ) + (s % HQ)) * NZB + 2 * pb) * w
                        if bi % 2 == 0:
                            nc.scalar.activation(Hbuf[:, hcol:hcol + 2 * w],
                                                 zpt[pb][:], RELU)
                        else:
                            nc.vector.tensor_scalar_max(
                                Hbuf[:, hcol:hcol + 2 * w], zpt[pb][:], 0.0)
                    act_banks = [zb for pb in act_pairs
                                 for zb in (2 * pb, 2 * pb + 1)]
                    yp = yps[s % 2]
                    for k, zb in enumerate(act_banks):
                        hcol = (((sg * HQ) + (s % HQ)) * NZB + zb) * w
                        nc.tensor.matmul(
                            yp[:, :],
                            WTS[:, WL2_OFF + zb * 128:WL2_OFF + (zb + 1) * 128],
                            Hbuf[:, hcol:hcol + w],
                            start=(k == 0), stop=(k == len(act_banks) - 1),
                            skip_group_check=True,
                            tile_position=(0, 0))
                    sl = sg * XR_S + (s % XR_S)
                    nc.vector.tensor_scalar_add(
                        X[:, sl * w:(sl + 1) * w], yp[:], B2C[:])
                    if s - 5 >= 0:
                        dma_out(sg, s - 5)
            for so in range(max(0, nsw - 5), nsw):
                for sg in range(NSTREAM):
                    dma_out(sg, so)
    return nc


# ------------------------------------------------------------- host runtime
#
# Two modes:
#   single — this process drives all 8 cores through its one axon
#            tunnel (~58 MB/s put / ~53 MB/s threaded get).
#   split  — a worker subprocess owns cores 4-7 with its OWN axon
#            tunnel; the two tunnels together reach ~80 MB/s put /
#            ~68 MB/s get aggregate, and host pack/gather also run in
#            two processes (no shared GIL).
# The cold call always produces its result via the single path, then
# tries to bring up split mode; any failure falls back to single.
_RT = {}
_RT_LOCK = threading.Lock()
_DBG = os.environ.get("KT_DEBUG", "0") == "1"
N_HALF = N_CORES // 2
B_HALF = B_FULL // 2


def _dbg(msg, t0):
    if _DBG:
        import sys
        print(f"[kt {os.getpid()}] {msg}: {time.time() - t0:.3f}s",
              file=sys.stderr, flush=True)


def _static_phase_nz():
    pnz = np.zeros((NTRIO, 5), bool)
    for tau in range(NTRIO):
        pnz[tau, 0] = True
        for j in range(1, 5):
            pnz[tau, j] = any(n - j >= 0 for n in trio_nodes(tau))
    return pnz


def _pool():
    pool = _RT.get("pool")
    if pool is None:
        from concurrent.futures import ThreadPoolExecutor
        pool = ThreadPoolExecutor(max_workers=2 * N_CORES)
        _RT["pool"] = pool
    return pool


def _make_state(core_lo, core_hi):
    """Build nc + jitted sharded executable for devices [core_lo, core_hi)."""
    import jax
    import jax.numpy as jnp
    from jax.sharding import Mesh, NamedSharding, PartitionSpec
    from jax.experimental.shard_map import shard_map
    import concourse.bass2jax as b2j
    import concourse.mybir as mybir

    n_loc = core_hi - core_lo
    nc = _RT.get("nc")
    if nc is None:
        t0 = time.time()
        nc = build_bass(_static_phase_nz(), w=W, c=C)
        _dbg("build_bass", t0)
        t0 = time.time()
        nc.compile()
        _dbg("nc.compile", t0)
        _RT["nc"] = nc
    b2j.install_neuronx_cc_hook()
    assert nc.dbg_addr is None, "built with debug=False"

    devs = jax.devices()[core_lo:core_hi]
    assert len(devs) == n_loc
    mesh = Mesh(np.asarray(devs), ("core",))
    sh_core = NamedSharding(mesh, PartitionSpec("core"))

    partition_name = (nc.partition_id_tensor.name
                      if nc.partition_id_tensor else None)
    in_names, out_names, out_avals = [], [], []
    for alloc in nc.m.functions[0].allocations:
        if not isinstance(alloc, mybir.MemoryLocationSet):
            continue
        name = alloc.memorylocations[0].name
        if alloc.kind == "ExternalInput":
            if name != partition_name:
                in_names.append(name)
        elif alloc.kind == "ExternalOutput":
            out_names.append(name)
            out_avals.append(jax.core.ShapedArray(
                tuple(alloc.tensor_shape), mybir.dt.np(alloc.dtype)))
    assert in_names == ["nin"] and out_names == ["gen0", "gen1"], (
        in_names, out_names)
    all_names = in_names + out_names + (
        [partition_name] if partition_name else [])

    def _body(*args):
        operands = list(args)
        if partition_name:
            operands.append(b2j.partition_id_tensor())
        outs = b2j._bass_exec_p.bind(
            *operands,
            out_avals=tuple(out_avals),
            in_names=tuple(all_names),
            out_names=tuple(out_names),
            lowering_input_output_aliases=(),
            sim_require_finite=True,
            sim_require_nnan=True,
            nc=nc,
        )
        return tuple(outs)

    jitted = jax.jit(
        shard_map(_body, mesh=mesh,
                  in_specs=(PartitionSpec("core"),) * 3,
                  out_specs=(PartitionSpec("core"),) * 2, check_rep=False),
        donate_argnums=(1, 2), keep_unused=True)

    zeros_fn = jax.jit(
        lambda: (jnp.zeros((n_loc * NN // 2, B_SHARD), jnp.bfloat16),
                 jnp.zeros((n_loc * NN // 2, B_SHARD), jnp.bfloat16)),
        out_shardings=(sh_core, sh_core))

    return dict(jax=jax, devs=devs, n_loc=n_loc, sh_core=sh_core,
                jitted=jitted, zeros_fn=zeros_fn, donate_next=None)


def _fill_nbuf(nbuf, noise_rows, wrows, pool):
    """nbuf [n, NIN_ROWS, B_SHARD] bf16 <- noise_rows [n*B_SHARD, 64] f32.

    (The host has a single CPU, so this is serial either way; the
    one-pass strided ml_dtypes cast is the fastest option measured.)
    """
    n = nbuf.shape[0]
    n4 = noise_rows.reshape(n, B_SHARD, NN)

    def _fill(i):
        nbuf[i, :NN, :] = n4[i].T      # f32 -> bf16 cast + transpose
        nbuf[i, NN:, :] = wrows
    list(pool.map(_fill, range(n)))


def _run_half(st, nbuf, gen_out, col_pool, timeout=1800.0):
    """One full device round for st's cores: put, scatter, exec, fetch.

    Writes results into gen_out [n_loc*B_SHARD, 64] (any f32 array view).
    A fetch timeout raises (leaving the stuck threads behind) so callers
    can fall back rather than hang.
    """
    jax = st["jax"]
    n_loc = st["n_loc"]
    t1 = time.time()
    g0 = jax.device_put(nbuf.reshape(n_loc * NIN_ROWS, B_SHARD),
                        st["devs"][0])
    nin_g = jax.device_put(g0, st["sh_core"])
    del g0
    don = st["donate_next"]
    st["donate_next"] = None
    if don is None:
        don = st["zeros_fn"]()
    out_g = st["jitted"](nin_g, *don)
    _dbg("dispatch", t1)

    t1 = time.time()
    shards = []
    for og in out_g:
        ss = sorted(og.addressable_shards,
                    key=lambda s: s.index[0].start or 0)
        assert len(ss) == n_loc
        shards.append(ss)

    def _fetch(k):
        h, i = divmod(k, n_loc)
        a = np.asarray(shards[h][i].data)    # [32, 32768] bf16
        gen_out[i * B_SHARD:(i + 1) * B_SHARD,
                h * NN // 2:(h + 1) * NN // 2] = a.T
    futs = [col_pool.submit(_fetch, k) for k in range(2 * n_loc)]
    deadline = time.time() + timeout
    for f in futs:
        f.result(timeout=max(1.0, deadline - time.time()))
    _dbg("exec+fetch+gather", t1)
    st["donate_next"] = out_g


# ----------------------------------------------------- worker subprocesses
# Main handles cores [0, CP); worker i (1-based) handles [i*CP, (i+1)*CP).
NPROC = max(1, min(N_CORES, int(os.environ.get("KT_NPROC", "2"))))
CP = N_CORES // NPROC
B_PROC = CP * B_SHARD


def _ipc_paths(tag):
    return (f"/dev/shm/kt_{tag}_in.raw", f"/dev/shm/kt_{tag}_w.raw",
            f"/dev/shm/kt_{tag}_out.raw")


def _worker_main(tag, idx):
    """Entry point of worker subprocess idx (cores idx*CP..(idx+1)*CP-1)."""
    import sys
    import ml_dtypes
    bfnp = ml_dtypes.bfloat16
    in_p, w_p, out_p = _ipc_paths(tag)
    r0 = idx * B_PROC
    in_mm = np.memmap(in_p, np.float32, "r",
                      shape=(B_FULL, NN))[r0:r0 + B_PROC]
    w_mm = np.memmap(w_p, np.float32, "r", shape=(WREG_LEN,))
    out_mm = np.memmap(out_p, np.float32, "r+",
                       shape=(B_FULL, NN))[r0:r0 + B_PROC]

    st = _make_state(idx * CP, (idx + 1) * CP)
    pool = _pool()
    nbuf = np.empty((CP, NIN_ROWS, B_SHARD), bfnp)
    nbuf[:] = 0
    print("COMPILED", flush=True)

    line = sys.stdin.readline()
    if line.strip() != "GO":
        return
    # warm the tunnel + executables with two dummy rounds
    scratch = np.empty((B_PROC, NN), np.float32)
    for _ in range(2):
        _run_half(st, nbuf, scratch, pool)
    print("READY", flush=True)

    wrows = np.zeros((NIN_WROWS * B_SHARD,), bfnp)
    for line in sys.stdin:
        cmd = line.strip()
        if cmd == "RUN":
            t0 = time.time()
            wrows.reshape(-1)[:WREG_LEN] = np.asarray(w_mm)
            _fill_nbuf(nbuf, in_mm, wrows.reshape(NIN_WROWS, B_SHARD), pool)
            _run_half(st, nbuf, out_mm, pool)
            _dbg(f"worker{idx} round", t0)
            print("DONE", flush=True)
        elif cmd == "QUIT":
            break


def _spawn_worker(tag, idx):
    """Start worker subprocess idx (non-blocking); returns handle dict."""
    import subprocess
    import sys
    import queue

    here = os.path.dirname(os.path.abspath(__file__))
    code = (f"import sys; sys.path.insert(0, {here!r}); "
            f"import kernel; kernel._worker_main({tag!r}, {idx})")
    stderr = (open(f"/tmp/kt_worker{idx}.log", "ab") if _DBG
              else subprocess.DEVNULL)
    proc = subprocess.Popen(
        [sys.executable, "-u", "-c", code],
        stdin=subprocess.PIPE, stdout=subprocess.PIPE, stderr=stderr,
        text=True)
    q = queue.Queue()

    def _reader():
        for ln in proc.stdout:
            q.put(ln.strip())
    threading.Thread(target=_reader, daemon=True).start()
    return dict(proc=proc, q=q, idx=idx)


def _worker_wait(wk, want, timeout):
    import queue
    deadline = time.time() + timeout
    while True:
        if wk["proc"].poll() is not None:
            raise RuntimeError(f"worker {wk['idx']} died")
        try:
            ln = wk["q"].get(timeout=min(1.0, max(0.05,
                                                  deadline - time.time())))
        except queue.Empty:
            if time.time() > deadline:
                raise TimeoutError(f"worker {wk['idx']}: no {want} "
                                   f"in {timeout}s")
            continue
        if ln == want:
            return
        if ln.startswith("ERR"):
            raise RuntimeError(ln)


def _kill_workers():
    for wk in _RT.get("wks", []):
        try:
            wk["proc"].kill()
        except Exception:
            pass


def _try_enable_split(nbuf8):
    """Bring up split mode during the cold call; swallow any failure.

    The first execution of each process's executable is serialized (GO
    one worker at a time) — N concurrent first-executions deadlock the
    axon terminal; steady-state concurrent rounds are fine.
    """
    try:
        wks = _RT["wks"]
        timeout = float(os.environ.get("KT_WORKER_TIMEOUT", "900"))
        for wk in wks:
            _worker_wait(wk, "COMPILED", timeout)
        for wk in wks:
            wk["proc"].stdin.write("GO\n")
            wk["proc"].stdin.flush()
            _worker_wait(wk, "READY", timeout)
        st_main = _make_state(0, CP)
        scratch = np.empty((B_PROC, NN), np.float32)
        pool = _pool()
        for _ in range(2):
            _run_half(st_main, nbuf8[:CP], scratch, pool)
        _RT["st_main"] = st_main
        _RT["mode"] = "split"
        _dbg("split mode enabled", time.time())
    except Exception as e:
        if _DBG:
            import sys
            print(f"[kt] split disabled: {e!r}", file=sys.stderr, flush=True)
        _kill_workers()
        _RT["mode"] = "single"


def kernel(**inputs):
    import ml_dtypes
    bfnp = ml_dtypes.bfloat16

    t_all = time.time()
    noise = np.asarray(inputs["noise"], np.float32)      # [B, 64]
    W1 = np.asarray(inputs["W1"], np.float32)
    b1 = np.asarray(inputs["b1"], np.float32)
    W2 = np.asarray(inputs["W2"], np.float32)
    b2 = np.asarray(inputs["b2"], np.float32)
    # parent_idx is structurally fixed (banded DAG) — masking is baked
    # into the packed weights; int dtype preserved implicitly (unused on
    # device).
    assert noise.shape == (B_FULL, NN), noise.shape

    t0 = time.time()
    wreg, _ = pack_weights(W1, b1, W2, b2)
    _dbg("pack_weights", t0)

    pool = _pool()
    mode = _RT.get("mode")

    wrows = np.zeros((NIN_WROWS * B_SHARD,), bfnp)
    wrows.reshape(-1)[:WREG_LEN] = wreg
    wrows = wrows.reshape(NIN_WROWS, B_SHARD)

    if mode is None:
        # ---------------- cold call: known-good single path + split setup
        with _RT_LOCK:
            split_wanted = (os.environ.get("KT_SINGLE", "0") != "1"
                            and NPROC > 1)
            if split_wanted and "wks" not in _RT:
                try:
                    tag = str(os.getpid())
                    in_p, w_p, out_p = _ipc_paths(tag)
                    for p, nbytes in ((in_p, B_FULL * NN * 4),
                                      (w_p, WREG_LEN * 4),
                                      (out_p, B_FULL * NN * 4)):
                        with open(p, "wb") as f:
                            f.truncate(nbytes)
                    _RT["in_mm"] = np.memmap(in_p, np.float32, "r+",
                                             shape=(B_FULL, NN))
                    _RT["w_mm"] = np.memmap(w_p, np.float32, "r+",
                                            shape=(WREG_LEN,))
                    _RT["out_mm"] = np.memmap(out_p, np.float32, "r",
                                              shape=(B_FULL, NN))
                    _RT["wks"] = [_spawn_worker(tag, i)
                                  for i in range(1, NPROC)]
                except Exception:
                    _RT.pop("wks", None)
            st8 = _make_state(0, N_CORES)
            _RT["st8"] = st8
            nbuf8 = np.empty((N_CORES, NIN_ROWS, B_SHARD), bfnp)
            _RT["nbuf8"] = nbuf8
            t0 = time.time()
            _fill_nbuf(nbuf8, noise, wrows, pool)
            _dbg("host pack buffer", t0)
            gen = np.empty((B_FULL, NN), np.float32)
            _run_half(st8, nbuf8, gen, pool)     # tunnel warm round
            _run_half(st8, nbuf8, gen, pool)
            if "wks" in _RT:
                _try_enable_split(nbuf8)
            else:
                _RT["mode"] = "single"
            if _RT["mode"] == "split":
                # one full split round so the first timed call is steady
                _split_round(noise, wreg, wrows, gen)
        _dbg("kernel total (cold)", t_all)
        return gen

    gen = np.empty((B_FULL, NN), np.float32)
    if mode == "split":
        try:
            _split_round(noise, wreg, wrows, gen)
            _dbg("kernel total", t_all)
            return gen
        except Exception as e:
            if _DBG:
                import sys
                print(f"[kt] split round failed ({e!r}); falling back",
                      file=sys.stderr, flush=True)
            _kill_workers()
            _RT["mode"] = "single"

    # ---------------- single mode
    nbuf8 = _RT["nbuf8"]
    t0 = time.time()
    _fill_nbuf(nbuf8, noise, wrows, pool)
    _dbg("host pack buffer", t0)
    _run_half(_RT["st8"], nbuf8, gen, pool)
    _dbg("kernel total", t_all)
    return gen


def _split_round(noise, wreg, wrows, gen):
    wks = _RT["wks"]
    pool = _pool()
    t0 = time.time()
    _RT["w_mm"][:] = wreg
    _RT["in_mm"][B_PROC:] = noise[B_PROC:]
    for wk in wks:
        wk["proc"].stdin.write("RUN\n")
        wk["proc"].stdin.flush()
    _dbg("worker handoff", t0)
    t0 = time.time()
    nbuf8 = _RT["nbuf8"]
    _fill_nbuf(nbuf8[:CP], noise[:B_PROC], wrows, pool)
    _dbg("host pack buffer", t0)
    _run_half(_RT["st_main"], nbuf8[:CP], gen[:B_PROC], pool)
    t0 = time.time()
    done_t = float(os.environ.get("KT_DONE_TIMEOUT", "60"))
    for wk in wks:
        _worker_wait(wk, "DONE", done_t)
    gen[B_PROC:] = _RT["out_mm"][B_PROC:]
    _dbg("worker join+copy", t0)
